# revision 21
# baseline (speedup 1.0000x reference)
"""BinaryLinear (binarized nn.Linear) on 8 Trainium2 NeuronCores.

Reference op:
    alpha = mean(|W|, axis=1)                # per-output-row scale
    BW    = sign(W) * alpha                  # sign(0) := +1
    Y     = einsum('bsi,oi->bso', X, BW) + bias

Distribution: data-parallel over the batch dim (8 batches -> 1 per core).
Each core receives its batch slice of X pre-transposed, split along the
contraction dim into an fp16 part (k 0..1279) and an fp8-e4m3 part
(k 1280..2047), the full weight as fp16 in two host-pretiled layouts
(wS: per-out-chunk stationary source for the sign, w: natural rows for
the alpha reduction), and bias f32. Each core computes the full
[tok, out] output for its batch element (stored transposed [out, tok]
fp16); the host casts back to f32, transposes and stacks.

Precision/speed tradeoff: the PE runs fp8 matmuls in DoubleRow mode at
2 contraction rows per cell-cycle, so the 6 fp8 k-chunks cost 3 matmuls
instead of 6. Binarized weights are exactly +-0.5 in BOTH fp16 and fp8
(the missing x2 folds into alpha2 = 2*mean|W|), and products +-0.5*x8
are exact in the PE's e10m10 lanes, so the only error is quantizing x:
fp16 on 10/16 chunks (~0.03%), e4m3 on 6/16 (measured 1.61e-2 total vs
the 2e-2 gate). wS is scaled x1024 on the host so near-zero weights
keep their sign in fp16 (only the sign is consumed).

Schedule (PE floor: 64 groups x (10 fp16 MM x 216 ns + 3 DR MM x ~244)
~ 185 us; everything else must hide under it). Both HWDGE rings share
HBM/SDMA bandwidth with no priority control, so ALL loads ride the sync
ring in priority order; the ACT ring carries only activations + output
stores:
  - sync order: pair-0/1 sign source (1 MiB), the 3 fp8 x tiles (the
    warmup's k-order starts with them; 512 KiB each), the 5 fp16 x
    chunk-pairs (1 MiB each), pair-1 signs, pair-0/1 alpha rows, then
    steady two-pair-ahead prefetch. Transfers overlap ~3-deep on the
    ring, so the gating chain for the first matmul is just
    wraw01 + bin + first fp8 tile.
  - a dummy activation pulls the one-time ACT_TABLE_LOAD (~1.3 us) off
    the first epilogue's critical path.
  - pair-0 epilogue is TWO-PASS: pass 1 copies psum -> SBUF f32 staging
    the moment each bank's accumulation stops (frees all 8 banks for
    pair-1 with zero PE stall, no alpha dependency); pass 2 applies
    alpha2*x+bias and stores once the alpha rows are reduced.
  - warmup: pair-0 out-chunks run with the k loop OUTERMOST (fp8
    double-chunks first, then fp16 pairs) so every arriving x tile
    unblocks matmuls on all 8 PSUM banks.
  - steady state: one psum group at a time so banks free staggered and
    epilogues overlap the next group's matmuls.
"""

import os

import numpy as np

B, T, K, O = 8, 2048, 2048, 2048  # batch, tokens, in_features, out_features
P = 128          # SBUF partitions
KC = K // P      # 16 k-chunks
KC16 = 10        # k-chunks carried in fp16
KC8 = KC - KC16  # k-chunks carried in fp8 (DoubleRow pairs)
D8 = KC8 // 2    # fp8 double-chunks
K16 = KC16 * P   # 1280
K8 = KC8 * P     # 768
OC = O // P      # 16 out-chunks
XG = KC16 // 2   # fp16 x chunk-pair groups
TN = 512         # moving free-dim per matmul
TT = T // TN     # 4 token tiles

N_CORES = 8

# Stashed by kernel() for test harnesses: BassKernelResults of the last run.
last_results = None

_cached_nc = None


def _build_program():
    global _cached_nc
    if _cached_nc is not None:
        return _cached_nc

    import concourse.tile as tile
    from concourse import bacc, bass_isa, mybir

    F32 = mybir.dt.float32
    F16 = mybir.dt.float16
    F8 = mybir.dt.float8e4
    F8E5 = mybir.dt.float8e5
    DR = mybir.MatmulPerfMode.DoubleRow
    IDENT = mybir.ActivationFunctionType.Identity
    ALU = mybir.AluOpType
    AX = mybir.AxisListType

    nc = bacc.Bacc("TRN2", target_bir_lowering=False, debug=False,
                   num_devices=N_CORES)

    xT = nc.dram_tensor("xT", [K16, T], F16, kind="ExternalInput").ap()
    x8T = nc.dram_tensor("x8T", [K8, T], F8, kind="ExternalInput").ap()
    # wS: host-pretiled stationary source, wS[oc, p, c*128+j] =
    # 1024*weight[oc*128+j, c*128+p] -- per-partition rows are 4 KiB
    # contiguous so o-chunk pairs load as efficient 1 MiB DMAs
    # sign source rides as fp8-e5m2 (x1024): e5m2 subnormals reach
    # 2^-16 so no weight's sign is lost to zero-rounding, max 57344 so
    # nothing saturates -- and sign DMA halves vs fp16
    wS = nc.dram_tensor("wS", [OC, P, K], F8E5, kind="ExternalInput").ap()
    # alpha source rows ride as fp8 (x64-scaled: sigma*64=2 keeps the
    # whole distribution in e4m3 normals; mean|.| error ~0.06%)
    w = nc.dram_tensor("w", [O, K], F8, kind="ExternalInput").ap()
    b = nc.dram_tensor("b", [O], F32, kind="ExternalInput").ap()
    yT = nc.dram_tensor("yT", [O, T], F16, kind="ExternalOutput").ap()

    xT_r = xT.rearrange("(g i p) t -> p g i t", p=P, i=2)
    x8T_r = x8T.rearrange("(d i p) t -> p d i t", p=P, i=2)
    wS_r2 = wS.rearrange("(q o) p k -> p q o k", o=2)
    w_r = w.rearrange("(q o p) k -> p q o k", o=2, p=P)

    with tile.TileContext(nc) as tc:
        with (
            tc.tile_pool(name="xpool", bufs=1) as xpool,
            tc.tile_pool(name="x8pool", bufs=1) as x8pool,
            tc.tile_pool(name="wpool", bufs=2) as wpool,
            tc.tile_pool(name="spool", bufs=6) as spool,
            tc.tile_pool(name="s8pool", bufs=6) as s8pool,
            tc.tile_pool(name="npool", bufs=2) as npool,
            tc.tile_pool(name="apool", bufs=12) as apool,
            tc.tile_pool(name="opool", bufs=3) as opool,
            tc.tile_pool(name="stpool", bufs=8) as stpool,
            tc.tile_pool(name="const", bufs=1) as const,
            tc.tile_pool(name="psum", bufs=8, space="PSUM") as psum,
        ):
            def bin16(src_ap, o):
                sw = spool.tile([P, KC16, P], F16, tag="sw", name=f"sw{o}")
                nc.vector.tensor_scalar(sw, src_ap, 0.0, 0.5,
                                        op0=ALU.is_ge, op1=ALU.subtract)
                return sw

            def bin8(src_ap, o):
                sw8 = s8pool.tile([P, KC8, P], F8, tag="sw8",
                                  name=f"sw8_{o}")
                nc.vector.tensor_scalar(sw8, src_ap, 0.0, 0.5,
                                        op0=ALU.is_ge, op1=ALU.subtract)
                return sw8

            def sign16_prep_o(o):
                """Load + binarize one o-chunk's fp16-part signs only
                (320 KiB) -- the minimal gate for its first matmuls."""
                wr = wpool.tile([P, KC16, P], F8E5, tag="wraw16",
                                name=f"wr16_{o}")
                nc.sync.dma_start(out=wr, in_=wS[o, :, :K16])
                return bin16(wr, o)

            def sign8_prep_o(o):
                """Load + binarize one o-chunk's fp8-part signs (192
                KiB); consumed at the END of each psum group."""
                wr = wpool.tile([P, KC8, P], F8E5, tag="wraw8",
                                name=f"wr8_{o}")
                nc.sync.dma_start(out=wr, in_=wS[o, :, K16:])
                return bin8(wr, o)

            def sign_prep_o(o):
                """Steady-state: one full 512 KiB load, two binarizes."""
                wraw = wpool.tile([P, K], F8E5, tag="wraw", name=f"wraw{o}")
                nc.sync.dma_start(out=wraw, in_=wS[o])
                sw = bin16(wraw[:, :K16], o)
                sw8 = bin8(wraw[:, K16:], o)
                return sw, sw8

            def sign_prep(pair):
                return [sign_prep_o(2 * pair), sign_prep_o(2 * pair + 1)]

            def alpha_prep(pair):
                """alpha2 = 2*mean|W_row| for both o-chunks of a pair."""
                wn = npool.tile([P, 2, K], F8, tag="wn", name=f"wn{pair}")
                nc.sync.dma_start(out=wn, in_=w_r[:, pair])
                a2s = []
                for j in range(2):
                    asum = apool.tile([P, 1], F32, tag="asum",
                                      name=f"as{2 * pair + j}")
                    nc.vector.tensor_reduce(asum, wn[:, j], axis=AX.X,
                                            op=ALU.add,
                                            apply_absolute_value=True)
                    alpha2 = apool.tile([P, 1], F32, tag="alpha2",
                                        name=f"al{2 * pair + j}")
                    nc.vector.tensor_scalar_mul(alpha2, asum, 2.0 / (K * 64.0))
                    a2s.append(alpha2)
                return a2s

            def weight_prep(pair):
                sws = sign_prep(pair)
                a2s = alpha_prep(pair)
                return [sws[0] + (a2s[0],), sws[1] + (a2s[1],)]

            # warmup is DMA-bound and k-ordered fp8-first: the first
            # matmul gates on just wr8_0 (96 KiB e5m2) + bin + x8 d0
            # (512 KiB). fp16 signs + x16 follow; alphas (ACT-only
            # consumers) ride dead last.
            sw8_0 = sign8_prep_o(0)
            sw8_1 = sign8_prep_o(1)
            x8_tiles = []
            bias_sb = None
            dummy = None
            for d in range(D8):
                x8t = x8pool.tile([P, 2, T], F8, tag=f"x8_{d}")
                nc.sync.dma_start(out=x8t, in_=x8T_r[:, d])
                x8_tiles.append(x8t)
                if d == 0:
                    bias_sb = const.tile([P, OC], F32)
                    nc.sync.dma_start(out=bias_sb,
                                      in_=b.rearrange("(c p) -> p c", p=P))
                    # dummy activation: pull the one-time ACT table load
                    # off the first epilogue's critical path
                    dummy = const.tile([P, 1], F16)
                    nc.scalar.activation(dummy, bias_sb[:, 0:1], IDENT)
            sw16_0 = sign16_prep_o(0)
            sw16_1 = sign16_prep_o(1)
            x_tiles = []
            for g in range(XG):
                xt = xpool.tile([P, 2, T], F16, tag=f"x{g}")
                nc.sync.dma_start(out=xt, in_=xT_r[:, g])
                x_tiles.append(xt)
            sw16_2 = sign16_prep_o(2)
            sw16_3 = sign16_prep_o(3)
            sw8_2 = sign8_prep_o(2)
            sw8_3 = sign8_prep_o(3)
            sw45 = sign_prep(2)
            a01 = alpha_prep(0)
            a23 = alpha_prep(1)
            a45 = alpha_prep(2)
            prepped = {0: [(sw16_0, sw8_0, a01[0]), (sw16_1, sw8_1, a01[1])],
                       1: [(sw16_2, sw8_2, a23[0]), (sw16_3, sw8_3, a23[1])],
                       2: [sw45[0] + (a45[0],), sw45[1] + (a45[1],)]}

            def rhs16(c, t):
                return x_tiles[c // 2][:, c % 2, t * TN:(t + 1) * TN]

            def rhs8(d, t):
                return x8_tiles[d][:, :, t * TN:(t + 1) * TN]

            def mm_group(ps_t, sw, sw8, t):
                for c in range(KC16):
                    nc.tensor.matmul(
                        ps_t, lhsT=sw[:, c, :], rhs=rhs16(c, t),
                        start=(c == 0), stop=False)
                for d in range(D8):
                    nc.tensor.matmul(
                        ps_t, lhsT=sw8[:, 2 * d:2 * d + 2, :], rhs=rhs8(d, t),
                        perf_mode=DR, start=False, stop=(d == D8 - 1))

            def epilogue(src, o, t, a2, name):
                ot = opool.tile([P, TN], F16, tag="ot", name=name)
                nc.scalar.activation(ot, src, IDENT,
                                     bias=bias_sb[:, o:o + 1], scale=a2)
                # stores ride the ACT HW-DGE ring: the sync ring's
                # in-order issue stream must stay pure loads, else weight
                # prefetch DMAs queue behind epilogue-gated stores
                nc.scalar.dma_start(
                    out=yT[o * P:(o + 1) * P, t * TN:(t + 1) * TN], in_=ot)

            for pair in range(OC // 2):
                o0, o1 = 2 * pair, 2 * pair + 1
                pair_w = prepped.pop(pair)
                ps = [psum.tile([P, TN], F32, tag="ps", name=f"ps{pair}_{i}")
                      for i in range(8)]

                if pair == 0:
                    # x still streaming in: k outermost, fp8 double-
                    # chunks FIRST (their x tiles lead the queue) so the
                    # PE starts ~3 us after main; fp16 chunks follow as
                    # their 1 MiB x pairs land
                    for d in range(D8):
                        for j in range(2):
                            sw8 = pair_w[j][1]
                            for t in range(TT):
                                nc.tensor.matmul(
                                    ps[j * TT + t],
                                    lhsT=sw8[:, 2 * d:2 * d + 2, :],
                                    rhs=rhs8(d, t),
                                    perf_mode=DR,
                                    start=(d == 0), stop=False)
                    for c in range(KC16):
                        for j in range(2):
                            sw = pair_w[j][0]
                            for t in range(TT):
                                nc.tensor.matmul(
                                    ps[j * TT + t],
                                    lhsT=sw[:, c, :], rhs=rhs16(c, t),
                                    start=False, stop=(c == KC16 - 1))
                    # two-pass epilogue: pass 1 parks each bank's psum in
                    # SBUF the moment it stops (banks free for pair-1
                    # with zero PE stall, no alpha dependency) ...
                    stage = []
                    for i in range(8):
                        st = stpool.tile([P, TN], F32, tag="st",
                                         name=f"st{i}")
                        nc.scalar.activation(st, ps[i], IDENT)
                        stage.append(st)
                    # ... pass 2 applies alpha2*x+bias and stores once
                    # the alphas (queued behind x) are ready
                    for j in range(2):
                        for t in range(TT):
                            epilogue(stage[j * TT + t], (o0, o1)[j], t,
                                     pair_w[j][2], f"ot{pair}_{j}_{t}")
                else:
                    # steady state: one psum group at a time so groups
                    # finish staggered -- banks free incrementally and
                    # epilogues overlap the next group's matmuls
                    for j in range(2):
                        for t in range(TT):
                            mm_group(ps[j * TT + t], pair_w[j][0],
                                     pair_w[j][1], t)
                            epilogue(ps[j * TT + t], (o0, o1)[j], t,
                                     pair_w[j][2], f"ot{pair}_{j}_{t}")

                # prefetch: signs three pairs out (they gate the PE),
                # alphas two pairs out (they only gate ACT epilogues)
                if pair + 3 < OC // 2:
                    sws = sign_prep(pair + 3)
                    a2s = alpha_prep(pair + 3)
                    prepped[pair + 3] = [sws[0] + (a2s[0],),
                                         sws[1] + (a2s[1],)]

    nc.compile()
    _cached_nc = nc
    return nc


def _make_in_maps(x, weight, bias):
    import ml_dtypes

    f16 = np.float16
    f8 = ml_dtypes.float8_e4m3  # TRN FP8_EXP4-compatible for |v| <= 240
    # pretiled stationary source: wS[oc, p, c*128+j] = weight[oc*128+j,
    # c*128+p], scaled x1024 so near-zero weights keep their sign in fp16
    # (only the sign is consumed); alpha comes from the unscaled copy w
    f8e5 = ml_dtypes.float8_e5m2
    wS = np.ascontiguousarray(
        (weight * 1024.0).reshape(OC, P, KC, P).transpose(0, 3, 2, 1)
        .reshape(OC, P, K)).astype(f8e5)
    w = np.ascontiguousarray(weight * 64.0).astype(f8)
    b = np.ascontiguousarray(bias)
    in_maps = []
    for core in range(N_CORES):
        xb = np.ascontiguousarray(x[core].T)  # [in, tok] f32
        in_maps.append({"xT": xb[:K16].astype(f16),
                        "x8T": xb[K16:].astype(f8),
                        "wS": wS, "w": w, "b": b})
    return in_maps


def _setup_trace_hooks():
    """Provide the antenv.axon_hooks NTFF hook missing from this image and
    skip the artifact bucket upload so trace=True works locally."""
    import sys
    import types

    try:
        from antenv.axon_hooks import get_axon_ntff_profile_hook  # noqa: F401
    except ImportError:
        mod = types.ModuleType("antenv.axon_hooks")
        _h = [None]
        mod.set_axon_ntff_profile_hook = lambda h: _h.__setitem__(0, h)
        mod.get_axon_ntff_profile_hook = lambda: _h[0]
        sys.modules["antenv.axon_hooks"] = mod
        import antenv

        antenv.axon_hooks = mod
        from trn_agent_boot.trn_boot import _ntff_profile_via_ctypes

        mod.set_axon_ntff_profile_hook(
            _ntff_profile_via_ctypes("/opt/axon/libaxon_pjrt.so"))

    import concourse.bass_utils as bu

    bu.upload_artifacts = lambda tmpdir: f"local://{tmpdir}"


def kernel(x: np.ndarray, weight: np.ndarray, bias: np.ndarray) -> np.ndarray:
    global last_results
    from concourse.bass_utils import run_bass_kernel_spmd

    x = np.asarray(x, dtype=np.float32)
    weight = np.asarray(weight, dtype=np.float32)
    bias = np.asarray(bias, dtype=np.float32)

    nc = _build_program()
    in_maps = _make_in_maps(x, weight, bias)
    trace = bool(int(os.environ.get("KERNEL_TRACE", "0")))
    trace_cores = None
    if trace:
        _setup_trace_hooks()
        tc_env = os.environ.get("KERNEL_TRACE_CORES", "")
        if tc_env:
            trace_cores = [int(c) for c in tc_env.split(",")]
    res = run_bass_kernel_spmd(nc, in_maps, list(range(N_CORES)), trace=trace,
                               trace_cores=trace_cores)
    last_results = res

    out = np.empty((B, T, O), dtype=np.float32)
    for core in range(N_CORES):
        out[core] = res.results[core]["yT"].T.astype(np.float32)
    return out


# revision 22
# speedup vs baseline: 1.1717x; 1.1717x over previous
"""BinaryLinear (binarized nn.Linear) on 8 Trainium2 NeuronCores.

Reference op:
    alpha = mean(|W|, axis=1)                # per-output-row scale
    BW    = sign(W) * alpha                  # sign(0) := +1
    Y     = einsum('bsi,oi->bso', X, BW) + bias

Distribution: data-parallel over the batch dim (8 batches -> 1 per core).
Each core receives its batch slice of X pre-transposed, split along the
contraction dim into an fp16 part (k 0..1279) and an fp8-e4m3 part
(k 1280..2047), the full weight as fp16 in two host-pretiled layouts
(wS: per-out-chunk stationary source for the sign, w: natural rows for
the alpha reduction), and bias f32. Each core computes the full
[tok, out] output for its batch element (stored transposed [out, tok]
fp16); the host casts back to f32, transposes and stacks.

Precision/speed tradeoff: the PE runs fp8 matmuls in DoubleRow mode at
2 contraction rows per cell-cycle, so the 6 fp8 k-chunks cost 3 matmuls
instead of 6. Binarized weights are exactly +-0.5 in BOTH fp16 and fp8
(the missing x2 folds into alpha2 = 2*mean|W|), and products +-0.5*x8
are exact in the PE's e10m10 lanes, so the only error is quantizing x:
fp16 on 10/16 chunks (~0.03%), e4m3 on 6/16 (measured 1.61e-2 total vs
the 2e-2 gate). wS is scaled x1024 on the host so near-zero weights
keep their sign in fp16 (only the sign is consumed).

Schedule (PE floor: 64 groups x (10 fp16 MM x 216 ns + 3 DR MM x ~244)
~ 185 us; everything else must hide under it). Both HWDGE rings share
HBM/SDMA bandwidth with no priority control, so ALL loads ride the sync
ring in priority order; the ACT ring carries only activations + output
stores:
  - sync order: pair-0/1 sign source (1 MiB), the 3 fp8 x tiles (the
    warmup's k-order starts with them; 512 KiB each), the 5 fp16 x
    chunk-pairs (1 MiB each), pair-1 signs, pair-0/1 alpha rows, then
    steady two-pair-ahead prefetch. Transfers overlap ~3-deep on the
    ring, so the gating chain for the first matmul is just
    wraw01 + bin + first fp8 tile.
  - a dummy activation pulls the one-time ACT_TABLE_LOAD (~1.3 us) off
    the first epilogue's critical path.
  - pair-0 epilogue is TWO-PASS: pass 1 copies psum -> SBUF f32 staging
    the moment each bank's accumulation stops (frees all 8 banks for
    pair-1 with zero PE stall, no alpha dependency); pass 2 applies
    alpha2*x+bias and stores once the alpha rows are reduced.
  - warmup: pair-0 out-chunks run with the k loop OUTERMOST (fp8
    double-chunks first, then fp16 pairs) so every arriving x tile
    unblocks matmuls on all 8 PSUM banks.
  - steady state: one psum group at a time so banks free staggered and
    epilogues overlap the next group's matmuls.
"""

import os

import numpy as np

B, T, K, O = 8, 2048, 2048, 2048  # batch, tokens, in_features, out_features
P = 128          # SBUF partitions
KC = K // P      # 16 k-chunks
KC16 = 10        # k-chunks carried in fp16
KC8 = KC - KC16  # k-chunks carried in fp8 (DoubleRow pairs)
D8 = KC8 // 2    # fp8 double-chunks
K16 = KC16 * P   # 1280
K8 = KC8 * P     # 768
OC = O // P      # 16 out-chunks
XG = KC16 // 2   # fp16 x chunk-pair groups
TN = 512         # moving free-dim per matmul
TT = T // TN     # 4 token tiles

N_CORES = 8

# Stashed by kernel() for test harnesses: BassKernelResults of the last run.
last_results = None

_cached_nc = None


def _build_program():
    global _cached_nc
    if _cached_nc is not None:
        return _cached_nc

    import concourse.tile as tile
    from concourse import bacc, bass_isa, mybir

    F32 = mybir.dt.float32
    F16 = mybir.dt.float16
    F8 = mybir.dt.float8e4
    F8E5 = mybir.dt.float8e5
    DR = mybir.MatmulPerfMode.DoubleRow
    IDENT = mybir.ActivationFunctionType.Identity
    ALU = mybir.AluOpType
    AX = mybir.AxisListType

    nc = bacc.Bacc("TRN2", target_bir_lowering=False, debug=False,
                   num_devices=N_CORES)

    xT = nc.dram_tensor("xT", [K16, T], F16, kind="ExternalInput").ap()
    x8T = nc.dram_tensor("x8T", [K8, T], F8, kind="ExternalInput").ap()
    # wS: host-pretiled stationary source, wS[oc, p, c*128+j] =
    # 1024*weight[oc*128+j, c*128+p] -- per-partition rows are 4 KiB
    # contiguous so o-chunk pairs load as efficient 1 MiB DMAs
    # sign source rides as fp8-e5m2 (x1024): e5m2 subnormals reach
    # 2^-16 so no weight's sign is lost to zero-rounding, max 57344 so
    # nothing saturates -- and sign DMA halves vs fp16
    wS = nc.dram_tensor("wS", [OC, P, K], F8E5, kind="ExternalInput").ap()
    # alpha source rows ride as fp8 (x64-scaled: sigma*64=2 keeps the
    # whole distribution in e4m3 normals; mean|.| error ~0.06%)
    w = nc.dram_tensor("w", [O, K], F8, kind="ExternalInput").ap()
    b = nc.dram_tensor("b", [O], F32, kind="ExternalInput").ap()
    yT = nc.dram_tensor("yT", [O, T], F16, kind="ExternalOutput").ap()

    xT_r = xT.rearrange("(g i p) t -> p g i t", p=P, i=2)
    x8T_r = x8T.rearrange("(d i p) t -> p d i t", p=P, i=2)
    wS_r2 = wS.rearrange("(q o) p k -> p q o k", o=2)
    w_r = w.rearrange("(q o p) k -> p q o k", o=2, p=P)

    with tile.TileContext(nc) as tc:
        with (
            tc.tile_pool(name="xpool", bufs=1) as xpool,
            tc.tile_pool(name="x8pool", bufs=1) as x8pool,
            tc.tile_pool(name="wpool", bufs=2) as wpool,
            tc.tile_pool(name="spool", bufs=6) as spool,
            tc.tile_pool(name="s8pool", bufs=6) as s8pool,
            tc.tile_pool(name="npool", bufs=2) as npool,
            tc.tile_pool(name="apool", bufs=12) as apool,
            tc.tile_pool(name="opool", bufs=3) as opool,
            tc.tile_pool(name="stpool", bufs=8) as stpool,
            tc.tile_pool(name="const", bufs=1) as const,
            tc.tile_pool(name="psum", bufs=8, space="PSUM") as psum,
        ):
            def bin16(src_ap, o):
                sw = spool.tile([P, KC16, P], F16, tag="sw", name=f"sw{o}")
                nc.vector.tensor_scalar(sw, src_ap, 0.0, 0.5,
                                        op0=ALU.is_ge, op1=ALU.subtract)
                return sw

            def bin8(src_ap, o):
                sw8 = s8pool.tile([P, KC8, P], F8, tag="sw8",
                                  name=f"sw8_{o}")
                nc.vector.tensor_scalar(sw8, src_ap, 0.0, 0.5,
                                        op0=ALU.is_ge, op1=ALU.subtract)
                return sw8

            def sign16_prep_o(o):
                """Load + binarize one o-chunk's fp16-part signs only
                (320 KiB) -- the minimal gate for its first matmuls."""
                wr = wpool.tile([P, KC16, P], F8E5, tag="wraw16",
                                name=f"wr16_{o}")
                nc.sync.dma_start(out=wr, in_=wS[o, :, :K16])
                return bin16(wr, o)

            def sign8_prep_o(o, eng=None):
                """Load + binarize one o-chunk's fp8-part signs (96
                KiB); consumed first in the warmup k-order."""
                wr = wpool.tile([P, KC8, P], F8E5, tag="wraw8",
                                name=f"wr8_{o}")
                (eng or nc.sync).dma_start(out=wr, in_=wS[o, :, K16:])
                return bin8(wr, o)

            def sign_prep_o(o):
                """Steady-state: one full 512 KiB load, two binarizes."""
                wraw = wpool.tile([P, K], F8E5, tag="wraw", name=f"wraw{o}")
                nc.sync.dma_start(out=wraw, in_=wS[o])
                sw = bin16(wraw[:, :K16], o)
                sw8 = bin8(wraw[:, K16:], o)
                return sw, sw8

            def sign_prep(pair):
                return [sign_prep_o(2 * pair), sign_prep_o(2 * pair + 1)]

            def alpha_prep(pair):
                """alpha2 = 2*mean|W_row| for both o-chunks of a pair."""
                wn = npool.tile([P, 2, K], F8, tag="wn", name=f"wn{pair}")
                nc.sync.dma_start(out=wn, in_=w_r[:, pair])
                a2s = []
                for j in range(2):
                    asum = apool.tile([P, 1], F32, tag="asum",
                                      name=f"as{2 * pair + j}")
                    nc.vector.tensor_reduce(asum, wn[:, j], axis=AX.X,
                                            op=ALU.add,
                                            apply_absolute_value=True)
                    alpha2 = apool.tile([P, 1], F32, tag="alpha2",
                                        name=f"al{2 * pair + j}")
                    nc.vector.tensor_scalar_mul(alpha2, asum, 2.0 / (K * 64.0))
                    a2s.append(alpha2)
                return a2s

            def weight_prep(pair):
                sws = sign_prep(pair)
                a2s = alpha_prep(pair)
                return [sws[0] + (a2s[0],), sws[1] + (a2s[1],)]

            # warmup is DMA-bound and k-ordered fp8-first: the first
            # matmul gates on just wr8_0 (96 KiB e5m2) + bin + x8 d0
            # (512 KiB). fp16 signs + x16 follow; alphas (ACT-only
            # consumers) ride dead last.
            # the first-matmul gate (pair-0/1 fp8 signs + x8 d0 + bias,
            # 0.7 MiB) rides the ACT ring, whose DMA path wakes ~2 us
            # before the sync ring's -- cross-ring HBM contention is
            # negligible for these few early transfers
            sw8_0 = sign8_prep_o(0, nc.scalar)
            sw8_1 = sign8_prep_o(1, nc.scalar)
            x8_tiles = []
            bias_sb = None
            dummy = None
            for d in range(D8):
                x8t = x8pool.tile([P, 2, T], F8, tag=f"x8_{d}")
                (nc.scalar if d == 0 else nc.sync).dma_start(
                    out=x8t, in_=x8T_r[:, d])
                x8_tiles.append(x8t)
                if d == 0:
                    bias_sb = const.tile([P, OC], F32)
                    nc.scalar.dma_start(out=bias_sb,
                                        in_=b.rearrange("(c p) -> p c", p=P))
                    # dummy activation: pull the one-time ACT table load
                    # off the first epilogue's critical path
                    dummy = const.tile([P, 1], F16)
                    nc.scalar.activation(dummy, bias_sb[:, 0:1], IDENT)
            sw16_0 = sign16_prep_o(0)
            sw16_1 = sign16_prep_o(1)
            x_tiles = []
            for g in range(XG):
                xt = xpool.tile([P, 2, T], F16, tag=f"x{g}")
                nc.sync.dma_start(out=xt, in_=xT_r[:, g])
                x_tiles.append(xt)
            sw16_2 = sign16_prep_o(2)
            sw16_3 = sign16_prep_o(3)
            sw8_2 = sign8_prep_o(2)
            sw8_3 = sign8_prep_o(3)
            sw45 = sign_prep(2)
            a01 = alpha_prep(0)
            a23 = alpha_prep(1)
            a45 = alpha_prep(2)
            prepped = {0: [(sw16_0, sw8_0, a01[0]), (sw16_1, sw8_1, a01[1])],
                       1: [(sw16_2, sw8_2, a23[0]), (sw16_3, sw8_3, a23[1])],
                       2: [sw45[0] + (a45[0],), sw45[1] + (a45[1],)]}

            def rhs16(c, t):
                return x_tiles[c // 2][:, c % 2, t * TN:(t + 1) * TN]

            def rhs8(d, t):
                return x8_tiles[d][:, :, t * TN:(t + 1) * TN]

            def mm_group(ps_t, sw, sw8, t):
                for c in range(KC16):
                    nc.tensor.matmul(
                        ps_t, lhsT=sw[:, c, :], rhs=rhs16(c, t),
                        start=(c == 0), stop=False)
                for d in range(D8):
                    nc.tensor.matmul(
                        ps_t, lhsT=sw8[:, 2 * d:2 * d + 2, :], rhs=rhs8(d, t),
                        perf_mode=DR, start=False, stop=(d == D8 - 1))

            def epilogue(src, o, t, a2, name):
                ot = opool.tile([P, TN], F16, tag="ot", name=name)
                nc.scalar.activation(ot, src, IDENT,
                                     bias=bias_sb[:, o:o + 1], scale=a2)
                # stores ride the ACT HW-DGE ring: the sync ring's
                # in-order issue stream must stay pure loads, else weight
                # prefetch DMAs queue behind epilogue-gated stores
                nc.scalar.dma_start(
                    out=yT[o * P:(o + 1) * P, t * TN:(t + 1) * TN], in_=ot)

            for pair in range(OC // 2):
                o0, o1 = 2 * pair, 2 * pair + 1
                pair_w = prepped.pop(pair)
                ps = [psum.tile([P, TN], F32, tag="ps", name=f"ps{pair}_{i}")
                      for i in range(8)]

                if pair == 0:
                    # x still streaming in: k outermost, fp8 double-
                    # chunks FIRST (their x tiles lead the queue) so the
                    # PE starts ~3 us after main; fp16 chunks follow as
                    # their 1 MiB x pairs land
                    for d in range(D8):
                        for j in range(2):
                            sw8 = pair_w[j][1]
                            for t in range(TT):
                                nc.tensor.matmul(
                                    ps[j * TT + t],
                                    lhsT=sw8[:, 2 * d:2 * d + 2, :],
                                    rhs=rhs8(d, t),
                                    perf_mode=DR,
                                    start=(d == 0), stop=False)
                    for c in range(KC16):
                        for j in range(2):
                            sw = pair_w[j][0]
                            for t in range(TT):
                                nc.tensor.matmul(
                                    ps[j * TT + t],
                                    lhsT=sw[:, c, :], rhs=rhs16(c, t),
                                    start=False, stop=(c == KC16 - 1))
                    # two-pass epilogue: pass 1 parks each bank's psum in
                    # SBUF the moment it stops (banks free for pair-1
                    # with zero PE stall, no alpha dependency) ...
                    stage = []
                    for i in range(8):
                        st = stpool.tile([P, TN], F32, tag="st",
                                         name=f"st{i}")
                        nc.scalar.activation(st, ps[i], IDENT)
                        stage.append(st)
                    # ... pass 2 applies alpha2*x+bias and stores once
                    # the alphas (queued behind x) are ready
                    for j in range(2):
                        for t in range(TT):
                            epilogue(stage[j * TT + t], (o0, o1)[j], t,
                                     pair_w[j][2], f"ot{pair}_{j}_{t}")
                else:
                    # steady state: one psum group at a time so groups
                    # finish staggered -- banks free incrementally and
                    # epilogues overlap the next group's matmuls
                    for j in range(2):
                        for t in range(TT):
                            mm_group(ps[j * TT + t], pair_w[j][0],
                                     pair_w[j][1], t)
                            epilogue(ps[j * TT + t], (o0, o1)[j], t,
                                     pair_w[j][2], f"ot{pair}_{j}_{t}")

                # prefetch: signs three pairs out (they gate the PE),
                # alphas two pairs out (they only gate ACT epilogues)
                if pair + 3 < OC // 2:
                    sws = sign_prep(pair + 3)
                    a2s = alpha_prep(pair + 3)
                    prepped[pair + 3] = [sws[0] + (a2s[0],),
                                         sws[1] + (a2s[1],)]

    nc.compile()
    _cached_nc = nc
    return nc


def _make_in_maps(x, weight, bias):
    import ml_dtypes

    f16 = np.float16
    f8 = ml_dtypes.float8_e4m3  # TRN FP8_EXP4-compatible for |v| <= 240
    # pretiled stationary source: wS[oc, p, c*128+j] = weight[oc*128+j,
    # c*128+p], scaled x1024 so near-zero weights keep their sign in fp16
    # (only the sign is consumed); alpha comes from the unscaled copy w
    f8e5 = ml_dtypes.float8_e5m2
    wS = np.ascontiguousarray(
        (weight * 1024.0).reshape(OC, P, KC, P).transpose(0, 3, 2, 1)
        .reshape(OC, P, K)).astype(f8e5)
    w = np.ascontiguousarray(weight * 64.0).astype(f8)
    b = np.ascontiguousarray(bias)
    in_maps = []
    for core in range(N_CORES):
        xb = np.ascontiguousarray(x[core].T)  # [in, tok] f32
        in_maps.append({"xT": xb[:K16].astype(f16),
                        "x8T": xb[K16:].astype(f8),
                        "wS": wS, "w": w, "b": b})
    return in_maps


def _setup_trace_hooks():
    """Provide the antenv.axon_hooks NTFF hook missing from this image and
    skip the artifact bucket upload so trace=True works locally."""
    import sys
    import types

    try:
        from antenv.axon_hooks import get_axon_ntff_profile_hook  # noqa: F401
    except ImportError:
        mod = types.ModuleType("antenv.axon_hooks")
        _h = [None]
        mod.set_axon_ntff_profile_hook = lambda h: _h.__setitem__(0, h)
        mod.get_axon_ntff_profile_hook = lambda: _h[0]
        sys.modules["antenv.axon_hooks"] = mod
        import antenv

        antenv.axon_hooks = mod
        from trn_agent_boot.trn_boot import _ntff_profile_via_ctypes

        mod.set_axon_ntff_profile_hook(
            _ntff_profile_via_ctypes("/opt/axon/libaxon_pjrt.so"))

    import concourse.bass_utils as bu

    bu.upload_artifacts = lambda tmpdir: f"local://{tmpdir}"


def kernel(x: np.ndarray, weight: np.ndarray, bias: np.ndarray) -> np.ndarray:
    global last_results
    from concourse.bass_utils import run_bass_kernel_spmd

    x = np.asarray(x, dtype=np.float32)
    weight = np.asarray(weight, dtype=np.float32)
    bias = np.asarray(bias, dtype=np.float32)

    nc = _build_program()
    in_maps = _make_in_maps(x, weight, bias)
    trace = bool(int(os.environ.get("KERNEL_TRACE", "0")))
    trace_cores = None
    if trace:
        _setup_trace_hooks()
        tc_env = os.environ.get("KERNEL_TRACE_CORES", "")
        if tc_env:
            trace_cores = [int(c) for c in tc_env.split(",")]
    res = run_bass_kernel_spmd(nc, in_maps, list(range(N_CORES)), trace=trace,
                               trace_cores=trace_cores)
    last_results = res

    out = np.empty((B, T, O), dtype=np.float32)
    for core in range(N_CORES):
        out[core] = res.results[core]["yT"].T.astype(np.float32)
    return out


# revision 23
# speedup vs baseline: 1.1792x; 1.0064x over previous
"""BinaryLinear (binarized nn.Linear) on 8 Trainium2 NeuronCores.

Reference op:
    alpha = mean(|W|, axis=1)                # per-output-row scale
    BW    = sign(W) * alpha                  # sign(0) := +1
    Y     = einsum('bsi,oi->bso', X, BW) + bias

Distribution: data-parallel over the batch dim (8 batches -> 1 per core).
Each core receives its batch slice of X pre-transposed, split along the
contraction dim into an fp16 part (k 0..1279) and an fp8-e4m3 part
(k 1280..2047), the full weight as fp16 in two host-pretiled layouts
(wS: per-out-chunk stationary source for the sign, w: natural rows for
the alpha reduction), and bias f32. Each core computes the full
[tok, out] output for its batch element (stored transposed [out, tok]
fp16); the host casts back to f32, transposes and stacks.

Precision/speed tradeoff: the PE runs fp8 matmuls in DoubleRow mode at
2 contraction rows per cell-cycle, so the 6 fp8 k-chunks cost 3 matmuls
instead of 6. Binarized weights are exactly +-0.5 in BOTH fp16 and fp8
(the missing x2 folds into alpha2 = 2*mean|W|), and products +-0.5*x8
are exact in the PE's e10m10 lanes, so the only error is quantizing x:
fp16 on 10/16 chunks (~0.03%), e4m3 on 6/16 (measured 1.61e-2 total vs
the 2e-2 gate). wS is scaled x1024 on the host so near-zero weights
keep their sign in fp16 (only the sign is consumed).

Schedule (PE floor: 64 groups x (10 fp16 MM x 216 ns + 3 DR MM x ~244)
~ 185 us; everything else must hide under it). Both HWDGE rings share
HBM/SDMA bandwidth with no priority control, so ALL loads ride the sync
ring in priority order; the ACT ring carries only activations + output
stores:
  - sync order: pair-0/1 sign source (1 MiB), the 3 fp8 x tiles (the
    warmup's k-order starts with them; 512 KiB each), the 5 fp16 x
    chunk-pairs (1 MiB each), pair-1 signs, pair-0/1 alpha rows, then
    steady two-pair-ahead prefetch. Transfers overlap ~3-deep on the
    ring, so the gating chain for the first matmul is just
    wraw01 + bin + first fp8 tile.
  - a dummy activation pulls the one-time ACT_TABLE_LOAD (~1.3 us) off
    the first epilogue's critical path.
  - pair-0 epilogue is TWO-PASS: pass 1 copies psum -> SBUF f32 staging
    the moment each bank's accumulation stops (frees all 8 banks for
    pair-1 with zero PE stall, no alpha dependency); pass 2 applies
    alpha2*x+bias and stores once the alpha rows are reduced.
  - warmup: pair-0 out-chunks run with the k loop OUTERMOST (fp8
    double-chunks first, then fp16 pairs) so every arriving x tile
    unblocks matmuls on all 8 PSUM banks.
  - steady state: one psum group at a time so banks free staggered and
    epilogues overlap the next group's matmuls.
"""

import os

import numpy as np

B, T, K, O = 8, 2048, 2048, 2048  # batch, tokens, in_features, out_features
P = 128          # SBUF partitions
KC = K // P      # 16 k-chunks
KC16 = 10        # k-chunks carried in fp16
KC8 = KC - KC16  # k-chunks carried in fp8 (DoubleRow pairs)
D8 = KC8 // 2    # fp8 double-chunks
K16 = KC16 * P   # 1280
K8 = KC8 * P     # 768
OC = O // P      # 16 out-chunks
XG = KC16 // 2   # fp16 x chunk-pair groups
TN = 512         # moving free-dim per matmul
TT = T // TN     # 4 token tiles

N_CORES = 8

# Stashed by kernel() for test harnesses: BassKernelResults of the last run.
last_results = None

_cached_nc = None


def _build_program():
    global _cached_nc
    if _cached_nc is not None:
        return _cached_nc

    import concourse.tile as tile
    from concourse import bacc, bass_isa, mybir

    F32 = mybir.dt.float32
    F16 = mybir.dt.float16
    F8 = mybir.dt.float8e4
    F8E5 = mybir.dt.float8e5
    DR = mybir.MatmulPerfMode.DoubleRow
    IDENT = mybir.ActivationFunctionType.Identity
    ALU = mybir.AluOpType
    AX = mybir.AxisListType

    nc = bacc.Bacc("TRN2", target_bir_lowering=False, debug=False,
                   num_devices=N_CORES)

    xT = nc.dram_tensor("xT", [K16, T], F16, kind="ExternalInput").ap()
    x8T = nc.dram_tensor("x8T", [K8, T], F8, kind="ExternalInput").ap()
    # wS: host-pretiled stationary source, wS[oc, p, c*128+j] =
    # 1024*weight[oc*128+j, c*128+p] -- per-partition rows are 4 KiB
    # contiguous so o-chunk pairs load as efficient 1 MiB DMAs
    # sign source rides as fp8-e5m2 (x1024): e5m2 subnormals reach
    # 2^-16 so no weight's sign is lost to zero-rounding, max 57344 so
    # nothing saturates -- and sign DMA halves vs fp16
    wS = nc.dram_tensor("wS", [OC, P, K], F8E5, kind="ExternalInput").ap()
    # alpha source rows ride as fp8 (x64-scaled: sigma*64=2 keeps the
    # whole distribution in e4m3 normals; mean|.| error ~0.06%)
    w = nc.dram_tensor("w", [O, K], F8, kind="ExternalInput").ap()
    b = nc.dram_tensor("b", [O], F32, kind="ExternalInput").ap()
    yT = nc.dram_tensor("yT", [O, T], F16, kind="ExternalOutput").ap()

    xT_r = xT.rearrange("(g i p) t -> p g i t", p=P, i=2)
    x8T_r = x8T.rearrange("(d i p) t -> p d i t", p=P, i=2)
    wS_r2 = wS.rearrange("(q o) p k -> p q o k", o=2)
    w_r = w.rearrange("(q o p) k -> p q o k", o=2, p=P)

    with tile.TileContext(nc) as tc:
        with (
            tc.tile_pool(name="xpool", bufs=1) as xpool,
            tc.tile_pool(name="x8pool", bufs=1) as x8pool,
            tc.tile_pool(name="wpool", bufs=2) as wpool,
            tc.tile_pool(name="spool", bufs=6) as spool,
            tc.tile_pool(name="s8pool", bufs=6) as s8pool,
            tc.tile_pool(name="npool", bufs=2) as npool,
            tc.tile_pool(name="apool", bufs=12) as apool,
            tc.tile_pool(name="opool", bufs=3) as opool,
            tc.tile_pool(name="stpool", bufs=8) as stpool,
            tc.tile_pool(name="const", bufs=1) as const,
            tc.tile_pool(name="psum", bufs=8, space="PSUM") as psum,
        ):
            def bin16(src_ap, o):
                sw = spool.tile([P, KC16, P], F16, tag="sw", name=f"sw{o}")
                nc.vector.tensor_scalar(sw, src_ap, 0.0, 0.5,
                                        op0=ALU.is_ge, op1=ALU.subtract)
                return sw

            def bin8(src_ap, o):
                sw8 = s8pool.tile([P, KC8, P], F8, tag="sw8",
                                  name=f"sw8_{o}")
                nc.vector.tensor_scalar(sw8, src_ap, 0.0, 0.5,
                                        op0=ALU.is_ge, op1=ALU.subtract)
                return sw8

            def sign16_prep_o(o):
                """Load + binarize one o-chunk's fp16-part signs only
                (320 KiB) -- the minimal gate for its first matmuls."""
                wr = wpool.tile([P, KC16, P], F8E5, tag="wraw16",
                                name=f"wr16_{o}")
                nc.sync.dma_start(out=wr, in_=wS[o, :, :K16])
                return bin16(wr, o)

            def sign8_prep_o(o):
                """Load + binarize one o-chunk's fp8-part signs (192
                KiB); consumed at the END of each psum group."""
                wr = wpool.tile([P, KC8, P], F8E5, tag="wraw8",
                                name=f"wr8_{o}")
                nc.sync.dma_start(out=wr, in_=wS[o, :, K16:])
                return bin8(wr, o)

            def sign_prep_o(o):
                """Steady-state: one full 512 KiB load, two binarizes."""
                wraw = wpool.tile([P, K], F8E5, tag="wraw", name=f"wraw{o}")
                nc.sync.dma_start(out=wraw, in_=wS[o])
                sw = bin16(wraw[:, :K16], o)
                sw8 = bin8(wraw[:, K16:], o)
                return sw, sw8

            def sign_prep(pair):
                return [sign_prep_o(2 * pair), sign_prep_o(2 * pair + 1)]

            def alpha_prep(pair):
                """alpha2 = 2*mean|W_row| for both o-chunks of a pair."""
                wn = npool.tile([P, 2, K], F8, tag="wn", name=f"wn{pair}")
                nc.sync.dma_start(out=wn, in_=w_r[:, pair])
                a2s = []
                for j in range(2):
                    asum = apool.tile([P, 1], F32, tag="asum",
                                      name=f"as{2 * pair + j}")
                    nc.vector.tensor_reduce(asum, wn[:, j], axis=AX.X,
                                            op=ALU.add,
                                            apply_absolute_value=True)
                    alpha2 = apool.tile([P, 1], F32, tag="alpha2",
                                        name=f"al{2 * pair + j}")
                    nc.vector.tensor_scalar_mul(alpha2, asum, 2.0 / (K * 64.0))
                    a2s.append(alpha2)
                return a2s

            def weight_prep(pair):
                sws = sign_prep(pair)
                a2s = alpha_prep(pair)
                return [sws[0] + (a2s[0],), sws[1] + (a2s[1],)]

            # warmup is DMA-bound and k-ordered fp8-first: the first
            # matmul gates on just wr8_0 (96 KiB e5m2) + bin + x8 d0
            # (512 KiB). fp16 signs + x16 follow; alphas (ACT-only
            # consumers) ride dead last.
            sw8_0 = sign8_prep_o(0)
            sw8_1 = sign8_prep_o(1)
            x8_tiles = []
            bias_sb = None
            dummy = None
            for d in range(D8):
                x8t = x8pool.tile([P, 2, T], F8, tag=f"x8_{d}")
                nc.sync.dma_start(out=x8t, in_=x8T_r[:, d])
                x8_tiles.append(x8t)
                if d == 0:
                    bias_sb = const.tile([P, OC], F32)
                    nc.sync.dma_start(out=bias_sb,
                                      in_=b.rearrange("(c p) -> p c", p=P))
                    # dummy activation: pull the one-time ACT table load
                    # off the first epilogue's critical path
                    dummy = const.tile([P, 1], F16)
                    nc.scalar.activation(dummy, bias_sb[:, 0:1], IDENT)
            sw16_0 = sign16_prep_o(0)
            sw16_1 = sign16_prep_o(1)
            x_tiles = []
            for g in range(XG):
                xt = xpool.tile([P, 2, T], F16, tag=f"x{g}")
                nc.sync.dma_start(out=xt, in_=xT_r[:, g])
                x_tiles.append(xt)
            sw16_2 = sign16_prep_o(2)
            sw16_3 = sign16_prep_o(3)
            sw8_2 = sign8_prep_o(2)
            sw8_3 = sign8_prep_o(3)
            sw45 = sign_prep(2)
            a01 = alpha_prep(0)
            a23 = alpha_prep(1)
            a45 = alpha_prep(2)
            prepped = {0: [(sw16_0, sw8_0, a01[0]), (sw16_1, sw8_1, a01[1])],
                       1: [(sw16_2, sw8_2, a23[0]), (sw16_3, sw8_3, a23[1])],
                       2: [sw45[0] + (a45[0],), sw45[1] + (a45[1],)]}

            def rhs16(c, t):
                return x_tiles[c // 2][:, c % 2, t * TN:(t + 1) * TN]

            def rhs8(d, t):
                return x8_tiles[d][:, :, t * TN:(t + 1) * TN]

            def mm_group(ps_t, sw, sw8, t):
                for c in range(KC16):
                    nc.tensor.matmul(
                        ps_t, lhsT=sw[:, c, :], rhs=rhs16(c, t),
                        start=(c == 0), stop=False)
                for d in range(D8):
                    nc.tensor.matmul(
                        ps_t, lhsT=sw8[:, 2 * d:2 * d + 2, :], rhs=rhs8(d, t),
                        perf_mode=DR, start=False, stop=(d == D8 - 1))

            def epilogue(src, o, t, a2, name):
                ot = opool.tile([P, TN], F16, tag="ot", name=name)
                nc.scalar.activation(ot, src, IDENT,
                                     bias=bias_sb[:, o:o + 1], scale=a2)
                # stores ride the ACT HW-DGE ring: the sync ring's
                # in-order issue stream must stay pure loads, else weight
                # prefetch DMAs queue behind epilogue-gated stores
                nc.scalar.dma_start(
                    out=yT[o * P:(o + 1) * P, t * TN:(t + 1) * TN], in_=ot)

            for pair in range(OC // 2):
                o0, o1 = 2 * pair, 2 * pair + 1
                pair_w = prepped.pop(pair)
                ps = [psum.tile([P, TN], F32, tag="ps", name=f"ps{pair}_{i}")
                      for i in range(8)]

                if pair == 0:
                    # x still streaming in: k outermost, fp8 double-
                    # chunks FIRST (their x tiles lead the queue) so the
                    # PE starts ~3 us after main; fp16 chunks follow as
                    # their 1 MiB x pairs land
                    for d in range(D8):
                        for j in range(2):
                            sw8 = pair_w[j][1]
                            for t in range(TT):
                                nc.tensor.matmul(
                                    ps[j * TT + t],
                                    lhsT=sw8[:, 2 * d:2 * d + 2, :],
                                    rhs=rhs8(d, t),
                                    perf_mode=DR,
                                    start=(d == 0), stop=False)
                    for c in range(KC16):
                        for j in range(2):
                            sw = pair_w[j][0]
                            for t in range(TT):
                                nc.tensor.matmul(
                                    ps[j * TT + t],
                                    lhsT=sw[:, c, :], rhs=rhs16(c, t),
                                    start=False, stop=(c == KC16 - 1))
                    # two-pass epilogue: pass 1 parks each bank's psum in
                    # SBUF the moment it stops (banks free for pair-1
                    # with zero PE stall, no alpha dependency) ...
                    stage = []
                    for i in range(8):
                        st = stpool.tile([P, TN], F32, tag="st",
                                         name=f"st{i}")
                        nc.scalar.activation(st, ps[i], IDENT)
                        stage.append(st)
                    # ... pass 2 applies alpha2*x+bias and stores once
                    # the alphas (queued behind x) are ready
                    for j in range(2):
                        for t in range(TT):
                            epilogue(stage[j * TT + t], (o0, o1)[j], t,
                                     pair_w[j][2], f"ot{pair}_{j}_{t}")
                else:
                    # steady state: one psum group at a time so groups
                    # finish staggered -- banks free incrementally and
                    # epilogues overlap the next group's matmuls
                    for j in range(2):
                        for t in range(TT):
                            mm_group(ps[j * TT + t], pair_w[j][0],
                                     pair_w[j][1], t)
                            epilogue(ps[j * TT + t], (o0, o1)[j], t,
                                     pair_w[j][2], f"ot{pair}_{j}_{t}")

                # prefetch: signs three pairs out (they gate the PE),
                # alphas two pairs out (they only gate ACT epilogues)
                if pair + 3 < OC // 2:
                    sws = sign_prep(pair + 3)
                    a2s = alpha_prep(pair + 3)
                    prepped[pair + 3] = [sws[0] + (a2s[0],),
                                         sws[1] + (a2s[1],)]

    nc.compile()
    _cached_nc = nc
    return nc


def _make_in_maps(x, weight, bias):
    import ml_dtypes

    f16 = np.float16
    f8 = ml_dtypes.float8_e4m3  # TRN FP8_EXP4-compatible for |v| <= 240
    # pretiled stationary source: wS[oc, p, c*128+j] = weight[oc*128+j,
    # c*128+p], scaled x1024 so near-zero weights keep their sign in fp16
    # (only the sign is consumed); alpha comes from the unscaled copy w
    f8e5 = ml_dtypes.float8_e5m2
    wS = np.ascontiguousarray(
        (weight * 1024.0).reshape(OC, P, KC, P).transpose(0, 3, 2, 1)
        .reshape(OC, P, K)).astype(f8e5)
    w = np.ascontiguousarray(weight * 64.0).astype(f8)
    b = np.ascontiguousarray(bias)
    in_maps = []
    for core in range(N_CORES):
        xb = np.ascontiguousarray(x[core].T)  # [in, tok] f32
        in_maps.append({"xT": xb[:K16].astype(f16),
                        "x8T": xb[K16:].astype(f8),
                        "wS": wS, "w": w, "b": b})
    return in_maps


def _setup_trace_hooks():
    """Provide the antenv.axon_hooks NTFF hook missing from this image and
    skip the artifact bucket upload so trace=True works locally."""
    import sys
    import types

    try:
        from antenv.axon_hooks import get_axon_ntff_profile_hook  # noqa: F401
    except ImportError:
        mod = types.ModuleType("antenv.axon_hooks")
        _h = [None]
        mod.set_axon_ntff_profile_hook = lambda h: _h.__setitem__(0, h)
        mod.get_axon_ntff_profile_hook = lambda: _h[0]
        sys.modules["antenv.axon_hooks"] = mod
        import antenv

        antenv.axon_hooks = mod
        from trn_agent_boot.trn_boot import _ntff_profile_via_ctypes

        mod.set_axon_ntff_profile_hook(
            _ntff_profile_via_ctypes("/opt/axon/libaxon_pjrt.so"))

    import concourse.bass_utils as bu

    bu.upload_artifacts = lambda tmpdir: f"local://{tmpdir}"


def kernel(x: np.ndarray, weight: np.ndarray, bias: np.ndarray) -> np.ndarray:
    global last_results
    from concourse.bass_utils import run_bass_kernel_spmd

    x = np.asarray(x, dtype=np.float32)
    weight = np.asarray(weight, dtype=np.float32)
    bias = np.asarray(bias, dtype=np.float32)

    nc = _build_program()
    in_maps = _make_in_maps(x, weight, bias)
    trace = bool(int(os.environ.get("KERNEL_TRACE", "0")))
    trace_cores = None
    if trace:
        _setup_trace_hooks()
        tc_env = os.environ.get("KERNEL_TRACE_CORES", "")
        if tc_env:
            trace_cores = [int(c) for c in tc_env.split(",")]
    res = run_bass_kernel_spmd(nc, in_maps, list(range(N_CORES)), trace=trace,
                               trace_cores=trace_cores)
    last_results = res

    out = np.empty((B, T, O), dtype=np.float32)
    for core in range(N_CORES):
        out[core] = res.results[core]["yT"].T.astype(np.float32)
    return out
